# revision 1
# baseline (speedup 1.0000x reference)
"""DreamAttention (GQA + RoPE + causal) on 8 trn2 NeuronCores.

Sharding: DP=2 over batch x sequence-parallel over q-tiles (no collectives).
Core c -> (batch b = c // 4, seq rank r = c % 4). Core r owns q-tiles
[r, 7-r, 8+r, 15-r] (128 rows each, ascending) — every core gets exactly 34
k-tile-blocks of causal attention work, so the load is perfectly balanced.
Each core computes ALL 28 heads for its 512 q rows and the FULL K/V
(redundantly, 4x) — that redundancy is far cheaper than a ReduceScatter of
o_proj partials over the (slow) inter-core links.

Per-core dataflow (all matmuls in fp32r = full-rate ~tf32 precision):
  - host supplies x^T tiles (D on partitions): xq (the core's q columns) and
    xt (full sequence, for K/V)
  - projections: QT [d, 28h, 512q], KT [d, 4kv, 2048], VT -> V via PE transpose
  - RoPE via SBUF->SBUF DMA partition rotation + DVE mul/add; host sends
    per-core-gathered cos/sin for q and full-sequence cos/sin for k
  - attention in transposed form per (head, k-tile): S^T[k, q-suffix] ->
    exp -> PV accumulates out^T[d, q]; the ascending q-tile layout makes the
    causally-live q columns a suffix, so each k-tile processes only [128, w]
    with w in {512, 384, 256, 128}; causality inside the suffix is enforced
    with a host-built additive mask; softmax sums via ones-matmul over a
    DVE-accumulated P^T running sum; normalization fused into the
    PSUM->SBUF move, which overwrites the spent Q slice
  - o_proj: attnT stationary, full Wo moving, accumulate over 28 head-chunks;
    output rows are core-owned -> DMA straight to the external output
Host reassembles the 8 cores' row-slices into the full [2, 2048, 3584] output.
"""

import math

import numpy as np

import concourse.bass as bass
import concourse.mybir as mybir
import concourse.tile as tile
from concourse import bacc
from concourse.bass_utils import run_bass_kernel_spmd
from concourse.masks import make_identity

F32 = mybir.dt.float32
F32R = mybir.dt.float32r

B, S, D = 2, 2048, 3584
H, KVH, HD = 28, 4, 128
ROPE_THETA = 1000000.0
GQ = H // KVH   # 7 q heads per kv head
DKT = D // 128  # 28 k-tiles over D
SC = 512        # s-chunk width for K/V projection
NSC = S // SC   # 4
NKT = S // 128  # 16 k tiles over sequence
NDC = 7         # output D chunks of 512
NQT = 4         # q-tiles owned per core
QW = NQT * 128  # 512 q columns per core
SCALE = 1.0 / math.sqrt(HD)


def _qtiles(r):
    """Ascending q-tile ids owned by seq-rank r; sum of (t+1) == 34 for all r."""
    return [r, 7 - r, 8 + r, 15 - r]


def _wof(kti):
    # Live-suffix width for k-tile kti. Rank-independent: every rank's
    # ascending tile list [t0<t1<t2<t3] satisfies t0<=3, 4<=t1<=7, 8<=t2<=11,
    # 12<=t3<=15, so #(tiles >= kti) == 4 - kti//4 for all ranks.
    return 128 * (4 - kti // 4)


_NC_CACHE = {}


def _build_nc(loop_n=1, phases="ABC"):
    key = ("nc", loop_n, phases)
    if key in _NC_CACHE:
        return _NC_CACHE[key]

    nc = bacc.Bacc("TRN2", target_bir_lowering=False, debug=False, num_devices=8)

    xq_d = nc.dram_tensor("xq", [DKT, 128, QW], F32R, kind="ExternalInput").ap()
    xt_d = nc.dram_tensor("xt", [NSC, DKT, 128, SC], F32R, kind="ExternalInput").ap()
    wq_d = nc.dram_tensor("wq", [H, 128, DKT, 128], F32R, kind="ExternalInput").ap()
    wkv_d = nc.dram_tensor(
        "wkv", [2 * KVH, 2, 128, DKT // 2, 128], F32R, kind="ExternalInput"
    ).ap()
    wo_d = nc.dram_tensor("wo", [NDC, DKT, 128, 512], F32R, kind="ExternalInput").ap()
    cosq_d = nc.dram_tensor("cosq", [128, QW], F32R, kind="ExternalInput").ap()
    sinq_d = nc.dram_tensor("sinq", [128, QW], F32R, kind="ExternalInput").ap()
    cosk_d = nc.dram_tensor("cosk", [128, S], F32R, kind="ExternalInput").ap()
    sink_d = nc.dram_tensor("sink", [128, S], F32R, kind="ExternalInput").ap()
    mask_d = nc.dram_tensor("mask", [NKT, 128, 128], F32, kind="ExternalInput").ap()
    out_d = nc.dram_tensor("out", [NQT, 128, D], F32, kind="ExternalOutput").ap()

    with tile.TileContext(nc) as tc:
        with (
            tc.tile_pool(name="persist", bufs=1) as persist,
            tc.tile_pool(name="ps_proj", bufs=2, space="PSUM") as ps_proj,
            tc.tile_pool(name="ps_s", bufs=3, space="PSUM") as ps_s,
            tc.tile_pool(name="ps_o", bufs=2, space="PSUM") as ps_o,
            tc.tile_pool(name="ps_sum", bufs=1, space="PSUM") as ps_sum,
        ):
            # qt doubles as the attention-output buffer: att(h) overwrites
            # qt[:, h, :] once head h's scores are done.
            qt = persist.tile([128, H, QW], F32R, name="qt")
            ident = persist.tile([128, 128], F32, name="ident")
            ones = persist.tile([128, 1], F32R, name="ones")
            ones_f = persist.tile([128, 1], F32, name="ones_f")

            make_identity(nc, ident)
            nc.vector.memset(ones_f, 1.0)
            nc.vector.tensor_copy(ones, ones_f)

            for _rep in range(loop_n):
                def rope(dst, cos_ap, sin_ap, width, tmp):
                    t = tmp[:, :width]
                    nc.gpsimd.dma_start(out=t[0:64, :], in_=dst[64:128, :])
                    nc.gpsimd.dma_start(out=t[64:128, :], in_=dst[0:64, :])
                    nc.vector.tensor_mul(t, t, sin_ap)
                    nc.vector.tensor_mul(dst, dst, cos_ap)
                    nc.vector.tensor_add(dst, dst, t)

                if "A" in phases:
                    # ---- Phase A1: Q projection + fused Q-RoPE ----
                    with (
                        tc.tile_pool(name="xqp", bufs=1) as xqp,
                        tc.tile_pool(name="wqp", bufs=3) as wqp,
                        tc.tile_pool(name="qtab", bufs=1) as qtab,
                        tc.tile_pool(name="qrtmp", bufs=3) as qrtmp,
                    ):
                        cosq = qtab.tile([128, QW], F32R, name="cosq")
                        sinq = qtab.tile([128, QW], F32R, name="sinq")
                        nc.scalar.dma_start(out=cosq, in_=cosq_d)
                        nc.scalar.dma_start(out=sinq, in_=sinq_d)
                        xq = xqp.tile([128, DKT, QW], F32R, name="xq")
                        nc.scalar.dma_start(
                            out=xq, in_=xq_d.rearrange("k p q -> p k q")
                        )
                        for ct in range(H):
                            wblk = wqp.tile([128, DKT, 128], F32R, name="wq")
                            nc.scalar.dma_start(out=wblk, in_=wq_d[ct])
                            psum = ps_proj.tile([128, QW], F32, name="pp")
                            for kti in range(DKT):
                                nc.tensor.matmul(
                                    psum,
                                    wblk[:, kti, :],
                                    xq[:, kti, :],
                                    start=(kti == 0),
                                    stop=(kti == DKT - 1),
                                )
                            nc.vector.tensor_copy(qt[:, ct, :], psum)
                            tmp = qrtmp.tile([128, QW], F32R, name="qrtmp")
                            rope(qt[:, ct, :], cosq, sinq, QW, tmp)

                kvp_cm = tc.tile_pool(name="kvp", bufs=1)
                kvp = kvp_cm.__enter__()
                kt_t = kvp.tile([128, KVH, S], F32R, name="kt")
                vn = kvp.tile([128, KVH, NKT, 128], F32R, name="vn")

                if "A" in phases:
                    # ---- Phase A2: K/V projection over the full sequence ----
                    with (
                        tc.tile_pool(name="xtp", bufs=30) as xtp,
                        tc.tile_pool(name="wkvp", bufs=2) as wkvp,
                        tc.tile_pool(name="vtp", bufs=1) as vtp,
                    ):
                        for sc in range(NSC):
                            xts = []
                            for kti in range(DKT):
                                xtile = xtp.tile([128, SC], F32R, name="xt")
                                nc.sync.dma_start(out=xtile, in_=xt_d[sc, kti])
                                xts.append(xtile)
                            vtc = vtp.tile([128, KVH, SC], F32, name="vtc")
                            for ct in range(2 * KVH):  # 0-3: K heads, 4-7: V
                                psum = ps_proj.tile([128, SC], F32, name="pp")
                                for hf in range(2):
                                    wblk = wkvp.tile(
                                        [128, DKT // 2, 128], F32R, name="wkv"
                                    )
                                    if ct < KVH:
                                        nc.sync.dma_start(
                                            out=wblk, in_=wkv_d[ct, hf]
                                        )
                                    else:
                                        nc.scalar.dma_start(
                                            out=wblk, in_=wkv_d[ct, hf]
                                        )
                                    for kti in range(DKT // 2):
                                        gkt = hf * (DKT // 2) + kti
                                        nc.tensor.matmul(
                                            psum,
                                            wblk[:, kti, :],
                                            xts[gkt],
                                            start=(gkt == 0),
                                            stop=(gkt == DKT - 1),
                                        )
                                if ct < KVH:
                                    dest = kt_t[:, ct, sc * SC : (sc + 1) * SC]
                                else:
                                    dest = vtc[:, ct - KVH, :]
                                nc.vector.tensor_copy(dest, psum)
                            # V^T -> V natural, per chunk (4 s-tiles x 4 heads)
                            for kv in range(KVH):
                                for sti in range(SC // 128):
                                    st = sc * (SC // 128) + sti
                                    ptr = ps_o.tile([128, QW], F32, name="po")
                                    nc.tensor.transpose(
                                        ptr[:, 0:128],
                                        vtc[:, kv, sti * 128 : (sti + 1) * 128],
                                        ident,
                                    )
                                    nc.vector.tensor_copy(
                                        vn[:, kv, st, :], ptr[:, 0:128]
                                    )

                    # ---- Phase A3: K-RoPE ----
                    with (
                        tc.tile_pool(name="ropetab", bufs=1) as ropetab,
                        tc.tile_pool(name="ropep", bufs=3) as ropep,
                    ):
                        cosk = ropetab.tile([128, S], F32R, name="cosk")
                        sink = ropetab.tile([128, S], F32R, name="sink")
                        nc.scalar.dma_start(out=cosk, in_=cosk_d)
                        nc.scalar.dma_start(out=sink, in_=sink_d)
                        for kv in range(KVH):
                            for c in range(NSC):
                                sl = slice(c * SC, (c + 1) * SC)
                                tmp = ropep.tile([128, 512], F32R, name="ropetmp")
                                rope(
                                    kt_t[:, kv, sl],
                                    cosk[:, sl],
                                    sink[:, sl],
                                    SC,
                                    tmp,
                                )

                if "B" in phases:
                    # ---- Phase B: attention, 28 heads on the core's 512 q ----
                    with (
                        tc.tile_pool(name="ptp", bufs=3) as ptp,
                        tc.tile_pool(name="smallp", bufs=2) as smallp,
                        tc.tile_pool(name="maskp", bufs=1) as maskp,
                    ):
                        mask_t = maskp.tile([128, NKT, 128], F32, name="mask_t")
                        nc.scalar.dma_start(
                            out=mask_t, in_=mask_d.rearrange("k p q -> p k q")
                        )
                        for h in range(H):
                            kv = h // GQ
                            psum_o = ps_o.tile([128, QW], F32, name="po")
                            psum_r = ps_sum.tile([1, QW], F32, name="psr")
                            for kti in range(NKT):
                                w = _wof(kti)
                                lo = QW - w
                                psum_s = ps_s.tile([128, QW], F32, name="pss")
                                nc.tensor.matmul(
                                    psum_s[:, lo:],
                                    kt_t[:, kv, kti * 128 : (kti + 1) * 128],
                                    qt[:, h, lo:],
                                    start=True,
                                    stop=True,
                                )
                                # only the first live block can be diagonal;
                                # the host mask is triangular there (or zero
                                # when this k-tile is not one of the core's
                                # q-tiles)
                                nc.vector.tensor_add(
                                    psum_s[:, lo : lo + 128],
                                    psum_s[:, lo : lo + 128],
                                    mask_t[:, kti, :],
                                )
                                pt = ptp.tile([128, QW], F32R, name="pt")
                                nc.scalar.activation(
                                    pt[:, lo:],
                                    psum_s[:, lo:],
                                    mybir.ActivationFunctionType.Exp,
                                )
                                nc.tensor.matmul(
                                    psum_o[:, lo:],
                                    vn[:, kv, kti, :],
                                    pt[:, lo:],
                                    start=(kti == 0),
                                    stop=(kti == NKT - 1),
                                )
                                nc.tensor.matmul(
                                    psum_r[:, lo:],
                                    ones,
                                    pt[:, lo:],
                                    start=(kti == 0),
                                    stop=(kti == NKT - 1),
                                )
                            rec = smallp.tile([1, QW], F32, name="rec")
                            nc.vector.reciprocal(rec, psum_r)
                            bcast = smallp.tile([128, QW], F32, name="bcast")
                            nc.gpsimd.partition_broadcast(bcast, rec)
                            # fused normalize + PSUM->SBUF, overwriting head
                            # h's spent q columns
                            nc.vector.tensor_mul(qt[:, h, :], psum_o, bcast)

                kvp_cm.__exit__(None, None, None)

                if "C" in phases:
                    # ---- Phase C: o_proj (full Wo); rows are core-owned ----
                    with (
                        tc.tile_pool(name="wop", bufs=56) as wop,
                        tc.tile_pool(name="outp", bufs=3) as outp,
                    ):
                        for dc in range(NDC):
                            wo_tiles = []
                            for ct in range(DKT):
                                wt = wop.tile([128, 512], F32R, name="wo")
                                if ct % 2 == 0:
                                    nc.sync.dma_start(out=wt, in_=wo_d[dc, ct])
                                else:
                                    nc.scalar.dma_start(out=wt, in_=wo_d[dc, ct])
                                wo_tiles.append(wt)
                            for q in range(NQT):
                                psum = ps_proj.tile([128, 512], F32, name="pp")
                                for ct in range(DKT):
                                    nc.tensor.matmul(
                                        psum,
                                        qt[:, ct, q * 128 : (q + 1) * 128],
                                        wo_tiles[ct],
                                        start=(ct == 0),
                                        stop=(ct == DKT - 1),
                                    )
                                ob = outp.tile([128, 512], F32, name="ob")
                                nc.vector.tensor_copy(ob, psum)
                                nc.sync.dma_start(
                                    out=out_d[q, :, dc * 512 : (dc + 1) * 512],
                                    in_=ob,
                                )

    nc.finalize()
    _NC_CACHE[key] = nc
    return nc


def _host_inputs(hidden_states, Wq, Wk, Wv, Wo):
    hidden = np.asarray(hidden_states, dtype=np.float32)
    Wq = np.asarray(Wq, dtype=np.float32) * np.float32(SCALE)
    Wk = np.asarray(Wk, dtype=np.float32)
    Wv = np.asarray(Wv, dtype=np.float32)
    Wo = np.asarray(Wo, dtype=np.float32)

    inv_freq = 1.0 / ROPE_THETA ** (np.arange(0, HD, 2, dtype=np.float32) / HD)
    t = np.arange(S, dtype=np.float32)
    freqs = np.outer(t, inv_freq)  # [S, 64]
    cos_t = np.cos(freqs.T)  # [64, S]
    sin_t = np.sin(freqs.T)
    cosk = np.concatenate([cos_t, cos_t], axis=0).astype(np.float32)  # [128, S]
    sink = np.concatenate([-sin_t, sin_t], axis=0).astype(np.float32)

    # shared weight layouts (identical for every core)
    wq = np.ascontiguousarray(
        Wq.reshape(DKT, 128, H, 128).transpose(2, 1, 0, 3)
    )  # [h, p, kt, c]
    wk4 = Wk.reshape(DKT, 128, KVH, 128)
    wv4 = Wv.reshape(DKT, 128, KVH, 128)
    wkv = np.empty((2 * KVH, 2, 128, DKT // 2, 128), np.float32)
    for ct in range(KVH):
        for hf in range(2):
            ktsl = slice(hf * (DKT // 2), (hf + 1) * (DKT // 2))
            wkv[ct, hf] = wk4[ktsl, :, ct, :].transpose(1, 0, 2)
            wkv[KVH + ct, hf] = wv4[ktsl, :, ct, :].transpose(1, 0, 2)
    wo = np.ascontiguousarray(
        Wo.reshape(DKT, 128, NDC, 512).transpose(2, 0, 1, 3)
    )  # [dc, ct, p, d]

    in_maps = []
    for core in range(8):
        b, r = core // 4, core % 4
        tiles = _qtiles(r)
        qpos = np.concatenate(
            [np.arange(t0 * 128, (t0 + 1) * 128) for t0 in tiles]
        )  # [512] ascending global q positions
        xq = np.ascontiguousarray(
            hidden[b][qpos].reshape(QW, DKT, 128).transpose(1, 2, 0)
        )  # [kt, p, q]
        xt = np.ascontiguousarray(
            hidden[b].reshape(NSC, SC, DKT, 128).transpose(0, 2, 3, 1)
        )
        cosq = np.ascontiguousarray(cosk[:, qpos])
        sinq = np.ascontiguousarray(sink[:, qpos])
        # mask[kt]: [128, 128] additive mask for the FIRST live block of the
        # suffix (columns QW-w .. QW-w+128). Triangular when that block's
        # q-tile equals kt (the diagonal), all-zero otherwise.
        mask = np.zeros((NKT, 128, 128), np.float32)
        for kti in range(NKT):
            lo = QW - _wof(kti)
            kk = kti * 128 + np.arange(128)[:, None]
            qq = qpos[None, lo : lo + 128]
            mask[kti] = np.where(kk <= qq, 0.0, -30000.0)
        in_maps.append(
            {
                "xq": xq,
                "xt": xt,
                "wq": wq,
                "wkv": wkv,
                "wo": wo,
                "cosq": cosq,
                "sinq": sinq,
                "cosk": cosk,
                "sink": sink,
                "mask": mask,
            }
        )
    return in_maps


def kernel(hidden_states, Wq, Wk, Wv, Wo, trace=False):
    nc = _build_nc()
    in_maps = _host_inputs(hidden_states, Wq, Wk, Wv, Wo)
    res = run_bass_kernel_spmd(nc, in_maps, list(range(8)), trace=trace)
    out = np.empty((B, S, D), dtype=np.float32)
    for core in range(8):
        b, r = core // 4, core % 4
        o = res.results[core]["out"]  # [NQT, 128, D]
        for j, t0 in enumerate(_qtiles(r)):
            out[b, t0 * 128 : (t0 + 1) * 128, :] = o[j]
    if trace:
        kernel.last_exec_time_ns = res.exec_time_ns
    return out



# revision 6
# speedup vs baseline: 1.1131x; 1.1131x over previous
"""DreamAttention (GQA + RoPE + causal) on 8 trn2 NeuronCores.

Sharding: DP=2 over batch x sequence-parallel over q-tiles (no collectives).
Core c -> (batch b = c // 4, seq rank r = c % 4). Core r owns q-tiles
[r, 7-r, 8+r, 15-r] (128 rows each, ascending) — every core gets exactly 34
k-tile-blocks of causal attention work, so the load is perfectly balanced.
Each core computes ALL 28 heads for its 512 q rows and the FULL K/V
(redundantly, 4x) — that redundancy is far cheaper than a ReduceScatter of
o_proj partials over the (slow) inter-core links.

All matmul operands are bf16 (fp32 PSUM accumulation). Per the TRN2 cost
model, bf16 streams 1 row/cycle at ANY width while fp32r drops to 1/4 rate
below 256-wide moving operands — and bf16 halves DMA bytes, SBUF footprint,
and doubles DVE throughput.

Per-core dataflow:
  - projections: QT [d, 28h, 512q], KT [d, 4kv, 2048], VT -> V via PE
    transpose; K-RoPE is fused into the K/V chunk loop (DVE work hides
    under the PE projection matmuls)
  - attention in transposed form per (head, k-tile): S^T[k, q-suffix] ->
    exp -> PV accumulates out^T[d, q]. Software-pipelined 4 deep: the PE
    issues S(kti+1..4) before PV(kti) so it never stalls waiting on the
    scalar-engine exp. The causal mask add runs on GpSimd (Pool) to keep
    DVE free for the exp-sum accumulation.
  - softmax denominator: DVE accumulates P^T tiles into acc[128, 512q];
    ONE ones-matmul per head reduces acc over partitions (vs. a
    full-width ones-matmul per k-tile = ~130us of PE time).
  - o_proj: attnT stationary, full Wo moving, accumulate over 28
    head-chunks; Wo tiles prefetch during attention; output rows are
    core-owned -> DMA straight to the external output.
Host reassembles the 8 cores' row-slices into the full [2, 2048, 3584] output.
"""

import math

import numpy as np
from ml_dtypes import bfloat16

import concourse.bass as bass
import concourse.mybir as mybir
import concourse.tile as tile
from concourse import bacc
from concourse.bass_utils import run_bass_kernel_spmd
from concourse.masks import make_identity

F32 = mybir.dt.float32
BF16 = mybir.dt.bfloat16

B, S, D = 2, 2048, 3584
H, KVH, HD = 28, 4, 128
ROPE_THETA = 1000000.0
GQ = H // KVH   # 7 q heads per kv head
DKT = D // 128  # 28 k-tiles over D
SC = 512        # s-chunk width for K/V projection
NSC = S // SC   # 4
NKT = S // 128  # 16 k tiles over sequence
NDC = 7         # output D chunks of 512
NQT = 4         # q-tiles owned per core
QW = NQT * 128  # 512 q columns per core
SCALE = 1.0 / math.sqrt(HD)
PVDEPTH = 4     # attention software-pipeline depth (S runs ahead of PV)


def _qtiles(r):
    """Ascending q-tile ids owned by seq-rank r; sum of (t+1) == 34 for all r."""
    return [r, 7 - r, 8 + r, 15 - r]


def _wof(kti):
    # Live-suffix width for k-tile kti. Rank-independent: every rank's
    # ascending tile list [t0<t1<t2<t3] satisfies t0<=3, 4<=t1<=7, 8<=t2<=11,
    # 12<=t3<=15, so #(tiles >= kti) == 4 - kti//4 for all ranks.
    return 128 * (4 - kti // 4)


_NC_CACHE = {}


def _build_nc():
    key = "nc"
    if key in _NC_CACHE:
        return _NC_CACHE[key]

    nc = bacc.Bacc("TRN2", target_bir_lowering=False, debug=False, num_devices=8)

    xq_d = nc.dram_tensor("xq", [DKT, 128, QW], BF16, kind="ExternalInput").ap()
    xt_d = nc.dram_tensor("xt", [NSC, DKT, 128, SC], BF16, kind="ExternalInput").ap()
    wq_d = nc.dram_tensor("wq", [H, 128, DKT, 128], BF16, kind="ExternalInput").ap()
    wkv_d = nc.dram_tensor(
        "wkv", [2 * KVH, 2, 128, DKT // 2, 128], BF16, kind="ExternalInput"
    ).ap()
    wo_d = nc.dram_tensor("wo", [NDC, DKT, 128, 512], BF16, kind="ExternalInput").ap()
    cosq_d = nc.dram_tensor("cosq", [128, QW], BF16, kind="ExternalInput").ap()
    sinq_d = nc.dram_tensor("sinq", [128, QW], BF16, kind="ExternalInput").ap()
    cosk_d = nc.dram_tensor("cosk", [128, S], BF16, kind="ExternalInput").ap()
    sink_d = nc.dram_tensor("sink", [128, S], BF16, kind="ExternalInput").ap()
    mask_d = nc.dram_tensor("mask", [NKT, 128, 128], BF16, kind="ExternalInput").ap()
    out_d = nc.dram_tensor("out", [NQT, 128, D], F32, kind="ExternalOutput").ap()

    with tile.TileContext(nc) as tc:
        with tc.tile_pool(name="persist", bufs=1) as persist:
            # qt doubles as the attention-output buffer: att(h) overwrites
            # qt[:, h, :] once head h's scores are done.
            qt = persist.tile([128, H, QW], BF16, name="qt")
            kt_t = persist.tile([128, KVH, S], BF16, name="kt")
            vn = persist.tile([128, KVH, NKT, 128], BF16, name="vn")
            ident = persist.tile([128, 128], BF16, name="ident")
            ones = persist.tile([128, 1], BF16, name="ones")

            make_identity(nc, ident)
            nc.vector.memset(ones, 1.0)

            def rope(dst, cos_ap, sin_ap, width, tmp):
                t = tmp[:, :width]
                nc.gpsimd.dma_start(out=t[0:64, :], in_=dst[64:128, :])
                nc.gpsimd.dma_start(out=t[64:128, :], in_=dst[0:64, :])
                nc.vector.tensor_mul(t, t, sin_ap)
                nc.vector.tensor_mul(dst, dst, cos_ap)
                nc.vector.tensor_add(dst, dst, t)

            # ---- Phase A1: Q projection + fused Q-RoPE ----
            with (
                tc.tile_pool(name="xqp", bufs=1) as xqp,
                tc.tile_pool(name="wqp", bufs=3) as wqp,
                tc.tile_pool(name="qtab", bufs=1) as qtab,
                tc.tile_pool(name="qrtmp", bufs=3) as qrtmp,
                tc.tile_pool(name="ps_a", bufs=3, space="PSUM") as ps_a,
            ):
                cosq = qtab.tile([128, QW], BF16, name="cosq")
                sinq = qtab.tile([128, QW], BF16, name="sinq")
                nc.scalar.dma_start(out=cosq, in_=cosq_d)
                nc.scalar.dma_start(out=sinq, in_=sinq_d)
                xq = xqp.tile([128, DKT, QW], BF16, name="xq")
                # per-kti loads so the first matmul starts after one tile
                for kti in range(DKT):
                    nc.scalar.dma_start(out=xq[:, kti, :], in_=xq_d[kti])
                for ct in range(H):
                    wblk = wqp.tile([128, DKT, 128], BF16, name="wq")
                    nc.sync.dma_start(out=wblk, in_=wq_d[ct])
                    psum = ps_a.tile([128, QW], F32, name="pp")
                    for kti in range(DKT):
                        nc.tensor.matmul(
                            psum,
                            wblk[:, kti, :],
                            xq[:, kti, :],
                            start=(kti == 0),
                            stop=(kti == DKT - 1),
                        )
                    nc.vector.tensor_copy(qt[:, ct, :], psum)
                    tmp = qrtmp.tile([128, QW], BF16, name="qrtmp")
                    rope(qt[:, ct, :], cosq, sinq, QW, tmp)

            # ---- Phase A2: K/V projection over the full sequence, with
            # K-RoPE fused in so the DVE work hides under the PE matmuls ----
            with (
                tc.tile_pool(name="ropetab", bufs=1) as ropetab,
                tc.tile_pool(name="ropep", bufs=3) as ropep,
                tc.tile_pool(name="xtp", bufs=56) as xtp,
                tc.tile_pool(name="wkvp", bufs=4) as wkvp,
                tc.tile_pool(name="vtp", bufs=2) as vtp,
                tc.tile_pool(name="ps_kv", bufs=2, space="PSUM") as ps_kv,
                tc.tile_pool(name="ps_tr", bufs=2, space="PSUM") as ps_tr,
            ):
                cosk = ropetab.tile([128, S], BF16, name="cosk")
                sink = ropetab.tile([128, S], BF16, name="sink")
                nc.scalar.dma_start(out=cosk, in_=cosk_d)
                nc.scalar.dma_start(out=sink, in_=sink_d)
                for sc in range(NSC):
                    sl = slice(sc * SC, (sc + 1) * SC)
                    xts = []
                    for kti in range(DKT):
                        xtile = xtp.tile([128, SC], BF16, name="xt")
                        nc.sync.dma_start(out=xtile, in_=xt_d[sc, kti])
                        xts.append(xtile)
                    vtc = vtp.tile([128, KVH, SC], BF16, name="vtc")
                    for ct in range(2 * KVH):  # 0-3: K heads, 4-7: V
                        psum = ps_kv.tile([128, SC], F32, name="pp")
                        for hf in range(2):
                            wblk = wkvp.tile(
                                [128, DKT // 2, 128], BF16, name="wkv"
                            )
                            if ct % 2 == 0:
                                nc.sync.dma_start(out=wblk, in_=wkv_d[ct, hf])
                            else:
                                nc.scalar.dma_start(out=wblk, in_=wkv_d[ct, hf])
                            for kti in range(DKT // 2):
                                gkt = hf * (DKT // 2) + kti
                                nc.tensor.matmul(
                                    psum,
                                    wblk[:, kti, :],
                                    xts[gkt],
                                    start=(gkt == 0),
                                    stop=(gkt == DKT - 1),
                                )
                        if ct < KVH:
                            nc.vector.tensor_copy(kt_t[:, ct, sl], psum)
                            tmp = ropep.tile([128, SC], BF16, name="ropetmp")
                            rope(
                                kt_t[:, ct, sl], cosk[:, sl], sink[:, sl], SC, tmp
                            )
                        else:
                            nc.vector.tensor_copy(vtc[:, ct - KVH, :], psum)
                    # V^T -> V natural, per chunk (4 s-tiles x 4 heads)
                    for kv in range(KVH):
                        for sti in range(SC // 128):
                            st = sc * (SC // 128) + sti
                            ptr = ps_tr.tile([128, 128], BF16, name="ptr")
                            nc.tensor.transpose(
                                ptr,
                                vtc[:, kv, sti * 128 : (sti + 1) * 128],
                                ident,
                            )
                            nc.vector.tensor_copy(vn[:, kv, st, :], ptr)

            # ---- Phase B+C share the wop pool so Wo prefetches during B ----
            with (
                tc.tile_pool(name="wop", bufs=56) as wop,
                tc.tile_pool(name="outp", bufs=3) as outp,
            ):
                wo_tiles = {}

                def load_wo(dc):
                    tl = []
                    for ct in range(DKT):
                        wt = wop.tile([128, 512], BF16, name="wo")
                        nc.sync.dma_start(out=wt, in_=wo_d[dc, ct])
                        tl.append(wt)
                    wo_tiles[dc] = tl

                load_wo(0)
                load_wo(1)

                # ---- Phase B: attention, 28 heads on the core's 512 q ----
                with (
                    tc.tile_pool(name="ptp", bufs=PVDEPTH + 1) as ptp,
                    tc.tile_pool(name="accp", bufs=2) as accp,
                    tc.tile_pool(name="smallp", bufs=2) as smallp,
                    tc.tile_pool(name="maskp", bufs=1) as maskp,
                    tc.tile_pool(name="ps_s", bufs=PVDEPTH + 1, space="PSUM") as ps_s,
                    tc.tile_pool(name="ps_o", bufs=2, space="PSUM") as ps_o,
                    tc.tile_pool(name="ps_r", bufs=1, space="PSUM") as ps_r,
                ):
                    mask_t = maskp.tile([128, NKT, 128], BF16, name="mask_t")
                    nc.scalar.dma_start(
                        out=mask_t, in_=mask_d.rearrange("k p q -> p k q")
                    )
                    for h in range(H):
                        kv = h // GQ
                        psum_o = ps_o.tile([128, QW], F32, name="po")
                        acc = accp.tile([128, QW], BF16, name="acc")
                        pend = []

                        def emit_pv(ent):
                            kti, lo, pt = ent
                            nc.tensor.matmul(
                                psum_o[:, lo:],
                                vn[:, kv, kti, :],
                                pt[:, lo:],
                                start=(kti == 0),
                                stop=(kti == NKT - 1),
                            )

                        for kti in range(NKT):
                            w = _wof(kti)
                            lo = QW - w
                            psum_s = ps_s.tile([128, QW], F32, name="pss")
                            nc.tensor.matmul(
                                psum_s[:, lo:],
                                kt_t[:, kv, kti * 128 : (kti + 1) * 128],
                                qt[:, h, lo:],
                                start=True,
                                stop=True,
                            )
                            pt = ptp.tile([128, QW], BF16, name="pt")
                            nc.scalar.activation(
                                pt[:, lo:],
                                psum_s[:, lo:],
                                mybir.ActivationFunctionType.Exp,
                            )
                            # only the first live block can be diagonal; the
                            # host 0/1 mask is triangular there (or all-zero
                            # when this k-tile is not one of the core's
                            # q-tiles). GpSimd can't touch PSUM, so the mask
                            # is applied multiplicatively post-exp in SBUF.
                            nc.gpsimd.tensor_mul(
                                pt[:, lo : lo + 128],
                                pt[:, lo : lo + 128],
                                mask_t[:, kti, :],
                            )
                            # running softmax-denominator partial on DVE
                            if kti == 0:
                                nc.vector.tensor_copy(acc, pt)
                            else:
                                nc.vector.tensor_add(
                                    acc[:, lo:], acc[:, lo:], pt[:, lo:]
                                )
                            pend.append((kti, lo, pt))
                            if len(pend) > PVDEPTH:
                                emit_pv(pend.pop(0))
                        while pend:
                            emit_pv(pend.pop(0))
                        # denominator: single partition-reduce matmul per head
                        psum_r = ps_r.tile([1, QW], F32, name="psr")
                        nc.tensor.matmul(psum_r, ones, acc, start=True, stop=True)
                        rec = smallp.tile([1, QW], F32, name="rec")
                        nc.vector.reciprocal(rec, psum_r)
                        bcast = smallp.tile([128, QW], F32, name="bcast")
                        nc.gpsimd.partition_broadcast(bcast, rec)
                        # fused normalize + PSUM->SBUF, overwriting head h's
                        # spent q columns
                        nc.vector.tensor_mul(qt[:, h, :], psum_o, bcast)

                # ---- Phase C: o_proj (full Wo); rows are core-owned ----
                with tc.tile_pool(name="ps_c", bufs=2, space="PSUM") as ps_c:
                    for dc in range(NDC):
                        if dc + 2 < NDC:
                            load_wo(dc + 2)
                        tl = wo_tiles.pop(dc)
                        for q in range(NQT):
                            psum = ps_c.tile([128, 512], F32, name="pp")
                            for ct in range(DKT):
                                nc.tensor.matmul(
                                    psum,
                                    qt[:, ct, q * 128 : (q + 1) * 128],
                                    tl[ct],
                                    start=(ct == 0),
                                    stop=(ct == DKT - 1),
                                )
                            ob = outp.tile([128, 512], F32, name="ob")
                            nc.vector.tensor_copy(ob, psum)
                            nc.scalar.dma_start(
                                out=out_d[q, :, dc * 512 : (dc + 1) * 512],
                                in_=ob,
                            )

    nc.finalize()
    _NC_CACHE[key] = nc
    return nc


def _host_inputs(hidden_states, Wq, Wk, Wv, Wo):
    hidden = np.asarray(hidden_states, dtype=np.float32)
    Wq = np.asarray(Wq, dtype=np.float32) * np.float32(SCALE)
    Wk = np.asarray(Wk, dtype=np.float32)
    Wv = np.asarray(Wv, dtype=np.float32)
    Wo = np.asarray(Wo, dtype=np.float32)

    inv_freq = 1.0 / ROPE_THETA ** (np.arange(0, HD, 2, dtype=np.float32) / HD)
    t = np.arange(S, dtype=np.float32)
    freqs = np.outer(t, inv_freq)  # [S, 64]
    cos_t = np.cos(freqs.T)  # [64, S]
    sin_t = np.sin(freqs.T)
    cosk = np.concatenate([cos_t, cos_t], axis=0).astype(bfloat16)  # [128, S]
    sink = np.concatenate([-sin_t, sin_t], axis=0).astype(bfloat16)

    # shared weight layouts (identical for every core)
    wq = np.ascontiguousarray(
        Wq.reshape(DKT, 128, H, 128).transpose(2, 1, 0, 3)
    ).astype(bfloat16)  # [h, p, kt, c]
    wk4 = Wk.reshape(DKT, 128, KVH, 128)
    wv4 = Wv.reshape(DKT, 128, KVH, 128)
    wkv = np.empty((2 * KVH, 2, 128, DKT // 2, 128), np.float32)
    for ct in range(KVH):
        for hf in range(2):
            ktsl = slice(hf * (DKT // 2), (hf + 1) * (DKT // 2))
            wkv[ct, hf] = wk4[ktsl, :, ct, :].transpose(1, 0, 2)
            wkv[KVH + ct, hf] = wv4[ktsl, :, ct, :].transpose(1, 0, 2)
    wkv = wkv.astype(bfloat16)
    wo = np.ascontiguousarray(
        Wo.reshape(DKT, 128, NDC, 512).transpose(2, 0, 1, 3)
    ).astype(bfloat16)  # [dc, ct, p, d]

    in_maps = []
    for core in range(8):
        b, r = core // 4, core % 4
        tiles = _qtiles(r)
        qpos = np.concatenate(
            [np.arange(t0 * 128, (t0 + 1) * 128) for t0 in tiles]
        )  # [512] ascending global q positions
        xq = np.ascontiguousarray(
            hidden[b][qpos].reshape(QW, DKT, 128).transpose(1, 2, 0)
        ).astype(bfloat16)  # [kt, p, q]
        xt = np.ascontiguousarray(
            hidden[b].reshape(NSC, SC, DKT, 128).transpose(0, 2, 3, 1)
        ).astype(bfloat16)
        cosq = np.ascontiguousarray(cosk[:, qpos])
        sinq = np.ascontiguousarray(sink[:, qpos])
        # mask[kt]: [128, 128] multiplicative 0/1 mask for the FIRST live
        # block of the suffix (columns QW-w .. QW-w+128). Triangular when
        # that block's q-tile equals kt (the diagonal), all-zero otherwise.
        mask = np.zeros((NKT, 128, 128), np.float32)
        for kti in range(NKT):
            lo = QW - _wof(kti)
            kk = kti * 128 + np.arange(128)[:, None]
            qq = qpos[None, lo : lo + 128]
            mask[kti] = np.where(kk <= qq, 1.0, 0.0)
        mask = mask.astype(bfloat16)
        in_maps.append(
            {
                "xq": xq,
                "xt": xt,
                "wq": wq,
                "wkv": wkv,
                "wo": wo,
                "cosq": cosq,
                "sinq": sinq,
                "cosk": cosk,
                "sink": sink,
                "mask": mask,
            }
        )
    return in_maps


def kernel(hidden_states, Wq, Wk, Wv, Wo, trace=False):
    nc = _build_nc()
    in_maps = _host_inputs(hidden_states, Wq, Wk, Wv, Wo)
    res = run_bass_kernel_spmd(nc, in_maps, list(range(8)), trace=trace)
    out = np.empty((B, S, D), dtype=np.float32)
    for core in range(8):
        b, r = core // 4, core % 4
        o = res.results[core]["out"]  # [NQT, 128, D]
        for j, t0 in enumerate(_qtiles(r)):
            out[b, t0 * 128 : (t0 + 1) * 128, :] = o[j]
    if trace:
        kernel.last_exec_time_ns = res.exec_time_ns
    return out
